# revision 38
# baseline (speedup 1.0000x reference)
"""Trainium2 Bass kernel: batched dense GAT (PyG GATConv, eval, concat heads).

Contract: kernel(**inputs) takes FULL inputs (numpy/jax arrays) and returns the
FULL output [B, N, H*C] float32. Internally shards across 8 NeuronCores:
core c handles graph b = c//2, target-node half j0 = (c%2)*1024.

Math (per graph):
  x = f @ W                       [N, H, C]
  a_src[i,h] = <x[i,h,:], att_src[h,:]>,  a_dst likewise
  logit[i,j,h] = leaky_relu(a_src[i,h] + a_dst[j,h], 0.2)  masked to -1e9
  alpha = softmax over i (sources);  out[j] = sum_i alpha * x[i]  (+bias, ELU)

Key restructure vs a direct port: softmax over i is invariant to adding any
per-column constant, and lrelu(s) = 0.2*s + 0.8*relu(s) with
s = a_src_i + a_dst_j, so the per-column 0.2*a_dst_j term cancels:

  alpha  propto  exp(0.2 a_src_i) * exp(0.8 relu(s))
       =  u_i * max(w_i z_j, 1)          (exp monotone, relu = max(s,0))
       =  uw_i * max(z_j, v_i)
  where  z = exp(0.8 a_dst), uw = exp(a_src), v = exp(-0.8 a_src).

So NO transcendental is needed on the N x N volume; per (head, i-block) the
device computes just
  t    = max(zb, v_i)          -- one tensor_scalar (bf16, single AP scalar)
  pe_t = t * mask01            -- one tensor_tensor (2x mode, bf16)
and PE-accumulates [num | den] = [x*uw | uw]^T pe_t per target column (the
uw_i factor rides in the stationary operand, applied by the stage-1 ACT
copy). The tiny rank-8 projections (a_src/a_dst -> uw, v, z) are folded
into host-side input prep along with the feature transpose + self-loop OR.
"""

import sys

for _p in ("/opt/trn_rl_repo",):
    if _p not in sys.path:
        sys.path.insert(0, _p)

import numpy as np
import ml_dtypes

# Problem dims (fixed by the task)
B, N, D = 4, 2048, 512
H, C = 8, 64
HC = H * C
NCORES = 8
NJ = 1024        # target columns per core
P = 128
NIB = N // P     # 16 source blocks
NJB = NJ // P    # 8 target blocks
KD = D // P      # 4 contraction blocks
FB = 66          # per-head x-tilde stride: 64 x-cols + ones col + pad (4B align)

BF16 = ml_dtypes.bfloat16

_PROG = None  # cached bass program


def _build_program():
    import concourse.bass as bass
    import concourse.mybir as mybir
    import concourse.tile as tile

    f32 = mybir.dt.float32
    bf16 = mybir.dt.bfloat16
    AF = mybir.ActivationFunctionType
    OP = mybir.AluOpType

    nc = bass.Bass("TRN2", target_bir_lowering=False, debug=False)

    fT_d = nc.dram_tensor("fT", [D, N], bf16, kind="ExternalInput").ap()
    W_d = nc.dram_tensor("W", [D, HC], bf16, kind="ExternalInput").ap()
    # mask / uwu arrive pre-packed [128, NIB*...] (i-block-major columns) so
    # each loads in a handful of large DMAs — descriptor issue on the SP
    # engine (~600ns each) is what actually gates the kernel's start-up.
    mask_d = nc.dram_tensor("mask", [P, NIB * NJ], bf16, kind="ExternalInput").ap()
    uwu_d = nc.dram_tensor("uwu", [P, NIB * 2 * H], f32, kind="ExternalInput").ap()
    z_d = nc.dram_tensor("z", [H, NJ], bf16, kind="ExternalInput").ap()
    bias_d = nc.dram_tensor("bias", [1, HC], f32, kind="ExternalInput").ap()
    ident_d = nc.dram_tensor("ident", [P, P], f32, kind="ExternalInput").ap()
    out_d = nc.dram_tensor("out", [NJ, HC], f32, kind="ExternalOutput").ap()

    NHP = H // 2  # head pairs

    with tile.TileContext(nc) as tc:
        with (
            tc.tile_pool(name="persist", bufs=1) as pp,
            tc.tile_pool(name="psum_bk", bufs=2, space="PSUM") as bkp,
            tc.tile_pool(name="psum_tr", bufs=2, space="PSUM") as trp,
            tc.tile_pool(name="work2", bufs=3) as wp,
        ):
            # ---------------- constants / persistent loads ----------------
            # DMA issue order: first the small tiles DVE needs to start
            # stage 2 (zb[0:2], uwu), then stage 1's fT/W, then the mask
            # stream; late-needed broadcasts (zb[2:], bias) last.
            ident = pp.tile([P, P], f32, name="ident")
            nc.sync.dma_start(ident[:], ident_d[:])

            # z rows broadcast across partitions (DRAM APs may carry a zero
            # partition step; SBUF APs may not)
            zb = [pp.tile([P, NJ], bf16, name=f"zb{h}") for h in range(H)]
            for h in range(2):
                nc.sync.dma_start(
                    zb[h][:], z_d[h:h + 1, :].to_broadcast((P, NJ))
                )

            uwu_t = pp.tile([P, NIB * 2 * H], f32, name="uwu")
            nc.sync.dma_start(uwu_t[:], uwu_d[:])

            def uwu_col(ib, col):
                c = ib * 2 * H + col
                return uwu_t[:, c:c + 1]

            # mask groups 0/1 first (stage 2's first tensor_tensors need
            # them before PE needs W/fT), then W, then fT quarters
            # interleaved with the remaining mask groups (large ones
            # amortize the ~600ns/descriptor SP issue cost).
            mask_t = pp.tile([P, NIB * NJ], bf16, name="mask")
            MG = [(0, 1), (1, 1), (2, 2), (4, 4), (8, 8)]  # (ib0, n_ibs)

            def emit_mask_dma(g):
                ib0, nib = MG[g]
                nc.sync.dma_start(
                    mask_t[:, ib0 * NJ:(ib0 + nib) * NJ],
                    mask_d[:, ib0 * NJ:(ib0 + nib) * NJ],
                )

            emit_mask_dma(0)
            emit_mask_dma(1)

            Wt = []
            fTt = []
            for kd in range(KD):
                w_t = pp.tile([P, HC], bf16, name=f"w_{kd}")
                nc.sync.dma_start(w_t[:], W_d[kd * P:(kd + 1) * P, :])
                Wt.append(w_t)
                fTt.append(pp.tile([P, N], bf16, name=f"ft_{kd}"))

            for g in range(4):
                if g > 0:
                    emit_mask_dma(g + 1)
                for kd in range(KD):
                    nc.sync.dma_start(
                        fTt[kd][:, g * 4 * P:(g + 1) * 4 * P],
                        fT_d[kd * P:(kd + 1) * P, g * 4 * P:(g + 1) * 4 * P],
                    )

            def mask_ap(ib):
                return mask_t[:, ib * NJ:(ib + 1) * NJ]

            for h in range(2, H):
                nc.sync.dma_start(
                    zb[h][:], z_d[h:h + 1, :].to_broadcast((P, NJ))
                )
            bias_b = pp.tile([P, HC], f32, name="bias_b")
            nc.sync.dma_start(bias_b[:], bias_d.to_broadcast((P, HC)))

            # persistent cross-stage tensors
            xt = [pp.tile([P, H, FB], bf16, name=f"xt{ib}") for ib in range(NIB)]
            zt = [pp.tile([P, HC], f32, name=f"z{jt}") for jt in range(NJB)]

            # junk target for PE touches shares the px PSUM ring slot 0;
            # it is dead once the touches ran, so the ring may recycle it.
            junk_ps = bkp.tile([1, 1], f32, name="junk_ps", tag="px")

            # PE instructions can carry only ONE sync wait in the walrus
            # lowering, and the PE's semaphore clock advances only via
            # its own waits. Touch every DMA-loaded tile PE will read
            # with a tiny 1x1 matmul (one wait each); real matmuls then
            # never need more than one wait.
            for t in (*Wt, *fTt, ident):
                nc.tensor.matmul(
                    junk_ps[:], t[0:1, 0:1], t[0:1, 0:1],
                    start=True, stop=True,
                )

            # ---------------- stage 1: x-tilde = [x*uw | uw] ----------------
            def emit_px(ib):
                px = bkp.tile([P, HC], f32, name="px", tag="px")
                for kd in range(KD):
                    nc.tensor.matmul(
                        px[:],
                        fTt[kd][:, ib * P:(ib + 1) * P],
                        Wt[kd][:],
                        start=(kd == 0),
                        stop=(kd == KD - 1),
                    )
                # uw denominator column (col 64 of each head) — on ACT to
                # keep DVE free for stage 2
                nc.scalar.activation(
                    xt[ib][:, :, 64:65],
                    uwu_t[:, ib * 2 * H:ib * 2 * H + H],
                    AF.Copy,
                )
                # x*uw via the ACT copy's free per-partition scale
                # (PSUM's only reader is ACT — keeps later PE reads at a
                # single sync wait).
                for h in range(H):
                    nc.scalar.activation(
                        xt[ib][:, h, 0:64],
                        px[:, h * C:(h + 1) * C],
                        AF.Copy,
                        scale=uwu_col(ib, h),
                    )

            # ------------ stage 2: attention + aggregation (head pairs) -----
            # Heads run in pairs so one tensor_tensor applies the mask to
            # both (the mask AP repeats via a stride-0 middle axis).
            def emit_pair_dve(hp, ib):
                h0 = 2 * hp
                t2 = wp.tile([P, 2, NJ], bf16, name="t2", tag="t", bufs=4)
                for hh in range(2):
                    # t = max(z_j, v_i);  uw_i * t == u_i * max(w_i z_j, 1)
                    nc.vector.tensor_scalar(
                        out=t2[:, hh, :],
                        in0=zb[h0 + hh][:],
                        scalar1=uwu_col(ib, H + h0 + hh),
                        scalar2=None,
                        op0=OP.max,
                    )
                pe2 = wp.tile([P, 2, NJ], bf16, name="pe2", tag="pe", bufs=12)
                nc.vector.tensor_mul(
                    pe2[:],
                    t2[:],
                    mask_ap(ib).unsqueeze(1).broadcast_to((P, 2, NJ)),
                )
                return pe2

            def emit_pair_mms(nts4, hp, ib, pe2):
                h0 = 2 * hp
                for hh in range(2):
                    for jc in range(2):
                        nc.tensor.matmul(
                            nts4[2 * hh + jc][:],
                            xt[ib][:, h0 + hh, 0:65],
                            pe2[:, hh, jc * 512:(jc + 1) * 512],
                            start=(ib == 0),
                            stop=(ib == NIB - 1),
                        )

            # stage 3 per target block: bias + ELU + store.
            # elu(z) = min(relu(z), exp(z) - 1) since exp(z)-1 >= z:
            # one GpSimd add (bias), two ACT ops, one DVE stt per block.
            def emit_stage3(jt):
                zx = wp.tile([P, HC], f32, name="zx", tag="zx", bufs=2)
                nc.gpsimd.tensor_add(zx[:], zt[jt][:], bias_b[:])
                ee = wp.tile([P, HC], f32, name="ee", tag="ee", bufs=2)
                nc.scalar.activation(ee[:], zx[:], AF.Exp)
                rl = wp.tile([P, HC], f32, name="rl", tag="rl", bufs=2)
                nc.scalar.activation(rl[:], zx[:], AF.Relu)
                of = wp.tile([P, HC], f32, name="of", tag="of", bufs=2)
                nc.vector.scalar_tensor_tensor(
                    out=of[:],
                    in0=ee[:],
                    scalar=-1.0,
                    in1=rl[:],
                    op0=OP.add,
                    op1=OP.min,
                )
                nc.sync.dma_start(out_d[jt * P:(jt + 1) * P, :], of[:])

            # tail part A: drain the pair's four PSUM chains to SBUF right
            # after the accumulation stops — this is all the next pair's
            # PSUM ring waits on.
            def emit_tail_A(nts4):
                sbs = []
                for jc in range(2):
                    for hh in range(2):
                        nt_sb = wp.tile([65, 512], f32, name="nt_sb",
                                        tag="ntsb", bufs=5)
                        nc.scalar.copy(nt_sb[:], nts4[2 * hh + jc][:])
                        sbs.append((jc, hh, nt_sb))
                return sbs

            # tail part B: transpose + normalize (+ stage 3 for the final
            # pair). Deferred into the NEXT pair's accumulation so the PE
            # transposes don't stall it.
            def emit_tail_B(sbs, hp, last=False):
                h0 = 2 * hp
                for k, (jc, hh, nt_sb) in enumerate(sbs):
                    for jq in range(4):
                        jt = jc * 4 + jq
                        ptq = trp.tile([P, 65], f32, name="ptq", tag="tr")
                        nc.tensor.transpose(
                            ptq[:],
                            nt_sb[:, jq * P:(jq + 1) * P],
                            ident[0:65, 0:65],
                        )
                        rec = wp.tile([P, 1], f32, name="rec", tag="rec")
                        nc.vector.reciprocal(rec[:], ptq[:, 64:65])
                        nc.scalar.activation(
                            zt[jt][:, (h0 + hh) * C:(h0 + hh + 1) * C],
                            ptq[:, 0:64],
                            AF.Copy,
                            scale=rec[:],
                        )
                    if last and k % 2 == 1:
                        # this jc group's 4 target blocks are complete —
                        # flush them without waiting for the other group
                        for jq in range(4):
                            emit_stage3(jc * 4 + jq)


            # hp=0 interleaves with stage 1 on the PE queue (px groups lag
            # two i-blocks ahead of the accumulation groups so the ACT
            # copies never stall the PE).
            nts4 = [
                bkp.tile([65, 512], f32, name=f"nt0_{k}", tag="nt", bufs=4)
                for k in range(4)
            ]
            pe2_q = []
            for ib in range(NIB):
                emit_px(ib)
                pe2_q.append(emit_pair_dve(0, ib))
                if ib >= 2:
                    emit_pair_mms(nts4, 0, ib - 2, pe2_q[ib - 2])
            for ib in (NIB - 2, NIB - 1):
                emit_pair_mms(nts4, 0, ib, pe2_q[ib])
            pe2_q = None
            pending_B = (emit_tail_A(nts4), 0, False)

            for hp in range(1, NHP):
                nts4 = [
                    bkp.tile([65, 512], f32, name=f"nt{hp}_{k}", tag="nt",
                             bufs=4)
                    for k in range(4)
                ]
                for ib in range(NIB):
                    pe2 = emit_pair_dve(hp, ib)
                    emit_pair_mms(nts4, hp, ib, pe2)
                    if ib == 2 and pending_B is not None:
                        emit_tail_B(*pending_B)
                        pending_B = None
                pending_B = (emit_tail_A(nts4), hp, hp == NHP - 1)
            emit_tail_B(*pending_B)



    _strip_redundant_pe_waits(nc)
    _split_excess_waits(nc)
    return nc


# empirical per-engine sync-wait budgets in the walrus CoreV3 lowering
_WAIT_BUDGET = {
    "EngineType.PE": 1,
    "EngineType.Activation": 1,
    "EngineType.DVE": 1,
    "EngineType.Pool": 0,
    "EngineType.SP": 1,
}


def _inst_budget(i, eng):
    return _WAIT_BUDGET.get(eng)


def _split_excess_waits(nc):
    """Instructions whose on_wait exceeds the engine's wait budget get the
    excess waits moved onto NoOp instructions inserted just before them in
    the same (in-order) engine queue."""
    import concourse.mybir as mybir

    fn = nc.m.functions[0]
    n = 0
    for blk in fn.blocks:
        insts = blk.instructions
        k = 0
        while k < len(insts):
            i = insts[k]
            eng = str(getattr(i, "engine", ""))
            si = getattr(i, "sync_info", None)
            budget = _inst_budget(i, eng)
            if si is None or budget is None or len(si.on_wait) <= budget:
                k += 1
                continue
            ws = list(si.on_wait)
            excess, keep = ws[: len(ws) - budget], ws[len(ws) - budget:]
            for w in excess:
                nop = mybir.InstNoOp(name=f"I-wsplit{n}", ins=[], outs=[])
                n += 1
                nop.engine = i.engine
                nop.sync_info = type(si)(on_wait=[w], on_update=[])
                insts.insert(k, nop)
                k += 1
            si.on_wait = keep
            i.sync_info = si
            k += 1


def _strip_redundant_pe_waits(nc):
    """walrus allows only ONE sync wait per PE instruction. Tile emits
    [bank-reader-sem, PE-self-sem] pairs on PSUM slot reuse even though the
    reader wait transitively implies the PE WAW wait (the reader itself
    waited for the PE chain). Compute, per instruction in scheduled order,
    the PE tick each semaphore value transitively certifies, and drop PE
    self-waits that are covered by a co-occurring wait."""
    fn = nc.m.functions[0]
    flat = [i for blk in fn.blocks for i in blk.instructions]

    def _merge(dst, src):
        for k, v in src.items():
            if dst.get(k, 0) < v:
                dst[k] = v

    # engine -> its own completion semaphore (each engine executes its
    # instruction stream strictly in order, so waits on the engine's own
    # sem are always satisfied at dispatch and can be dropped)
    self_sem = {}
    for i in flat:
        si = getattr(i, "sync_info", None)
        eng = str(getattr(i, "engine", ""))
        if si is None or type(i).__name__ in ("InstNop", "InstDrain"):
            continue
        if eng not in self_sem and si.on_update:
            nm = si.on_update[0].ant_name
            if not nm.startswith(("DMAHW", "DMASW", "barrier")):
                self_sem[eng] = nm

    obs = {}        # engine -> observed vector clock {sem: tick}
    events = {}     # (sem, value) -> vector clock certified when sem hit value
    sem_val = {}
    for i in flat:
        eng = str(getattr(i, "engine", ""))
        si = getattr(i, "sync_info", None)
        if si is None:
            continue
        o = obs.setdefault(eng, {})
        for w in si.on_wait:
            if w.wait_value is None:
                continue
            if o.get(w.ant_name, 0) < w.wait_value:
                o[w.ant_name] = w.wait_value
            _merge(o, events.get((w.ant_name, w.wait_value), {}))
        if any(w.ant_name == self_sem.get(eng) for w in si.on_wait):
            si.on_wait = [
                w for w in si.on_wait if w.ant_name != self_sem.get(eng)
            ]
            i.sync_info = si
        if len(si.on_wait) > 1:
            ws = [w for w in si.on_wait]
            certs = []
            for w in ws:
                c = dict(events.get((w.ant_name, w.wait_value), {})) \
                    if w.wait_value is not None else {}
                if w.wait_value is not None:
                    c[w.ant_name] = max(c.get(w.ant_name, 0), w.wait_value)
                certs.append(c)
            # greedily keep waits not covered by the union of kept certs
            order = sorted(range(len(ws)), key=lambda j: -len(certs[j]))
            kept, covered = [], {}
            for j in order:
                w = ws[j]
                if (
                    w.wait_value is not None
                    and covered.get(w.ant_name, 0) >= w.wait_value
                ):
                    continue
                kept.append(j)
                _merge(covered, certs[j])
            if len(kept) < len(ws):
                si.on_wait = [ws[j] for j in sorted(kept)]
                i.sync_info = si
        for u in si.on_update:
            if u.update_value is None:
                continue
            v1 = sem_val.get(u.ant_name, 0) + u.update_value
            sem_val[u.ant_name] = v1
            cert = dict(o)
            cert[u.ant_name] = max(cert.get(u.ant_name, 0), v1)
            for vv in range(v1 - u.update_value + 1, v1 + 1):
                events[(u.ant_name, vv)] = cert
            if o.get(u.ant_name, 0) < v1:
                o[u.ant_name] = v1


def _get_program():
    global _PROG
    if _PROG is None:
        _PROG = _build_program()
    return _PROG


def _make_in_maps(features_batch, adj_mats_batch, W, att_src, att_dst, bias):
    f = np.asarray(features_batch, dtype=np.float32)
    adj = np.asarray(adj_mats_batch, dtype=np.int32)
    Wn = np.ascontiguousarray(np.asarray(W, dtype=np.float32))
    asv = np.asarray(att_src, dtype=np.float32)
    adv = np.asarray(att_dst, dtype=np.float32)
    bv = np.ascontiguousarray(np.asarray(bias, dtype=np.float32).reshape(1, HC))

    # fold the rank-8 attention projections into input prep:
    # a_src = f @ (W.reshape(D,H,C) . att_src), likewise a_dst
    Wa_src = np.einsum("dhc,hc->dh", Wn.reshape(D, H, C), asv)
    Wa_dst = np.einsum("dhc,hc->dh", Wn.reshape(D, H, C), adv)

    ident = np.eye(P, dtype=np.float32)
    W_bf = np.ascontiguousarray(Wn.astype(BF16))

    fT_bf = []
    uwu_all = []
    adst_all = []
    for b in range(B):
        fT_bf.append(np.ascontiguousarray(f[b].T.astype(BF16)))
        asrc = f[b] @ Wa_src  # [N, H] f32
        uwu = np.concatenate(
            [np.exp(asrc), np.exp(-0.8 * asrc)], axis=1
        ).astype(np.float32)
        # pack i-block-major: [128, NIB*2H]
        uwu_all.append(np.ascontiguousarray(
            uwu.reshape(NIB, P, 2 * H).transpose(1, 0, 2).reshape(P, NIB * 2 * H)
        ))
        adst_all.append(f[b] @ Wa_dst)  # [N, H] f32

    in_maps = []
    jdx = np.arange(NJ)
    for c in range(NCORES):
        b, half = divmod(c, 2)
        j0 = half * NJ
        m = (adj[b][:, j0:j0 + NJ] != 0)
        m[j0 + jdx, jdx] = True  # self-loops always present
        # pack i-block-major: [128, NIB*NJ]
        mask_bf = np.ascontiguousarray(
            m.astype(BF16).reshape(NIB, P, NJ).transpose(1, 0, 2)
            .reshape(P, NIB * NJ)
        )
        z = np.ascontiguousarray(
            np.exp(0.8 * adst_all[b][j0:j0 + NJ]).T.astype(BF16)
        )  # [H, NJ]
        in_maps.append(
            {
                "fT": fT_bf[b],
                "W": W_bf,
                "mask": mask_bf,
                "uwu": uwu_all[b],
                "z": z,
                "bias": bv,
                "ident": ident,
            }
        )
    return in_maps


_RUNNER = None  # cached (jitted_fn, in_names, out_names, n_params, zero_outs)


def _get_runner():
    """Build a jitted shard_map runner for the bass program (mirrors
    concourse.bass2jax.run_bass_via_pjrt but without output donation, so
    device-resident inputs can be reused across timed iterations)."""
    global _RUNNER
    if _RUNNER is not None:
        return _RUNNER
    import jax
    import concourse.mybir as mybir
    from concourse import bass2jax
    from jax.sharding import Mesh, PartitionSpec
    from jax.experimental.shard_map import shard_map

    bass2jax.install_neuronx_cc_hook()
    nc = _get_program()

    partition_name = (
        nc.partition_id_tensor.name if nc.partition_id_tensor else None
    )
    in_names, out_names, out_avals, zero_outs = [], [], [], []
    for alloc in nc.m.functions[0].allocations:
        if not isinstance(alloc, mybir.MemoryLocationSet):
            continue
        name = alloc.memorylocations[0].name
        if alloc.kind == "ExternalInput":
            if name != partition_name:
                in_names.append(name)
        elif alloc.kind == "ExternalOutput":
            shape = tuple(alloc.tensor_shape)
            dtype = mybir.dt.np(alloc.dtype)
            out_names.append(name)
            out_avals.append(jax.core.ShapedArray(shape, dtype))
            zero_outs.append(np.zeros(shape, dtype))
    n_params = len(in_names)
    all_names = in_names + out_names
    if partition_name is not None:
        all_names = all_names + [partition_name]

    def _body(*args):
        operands = list(args)
        if partition_name is not None:
            operands.append(bass2jax.partition_id_tensor())
        outs = bass2jax._bass_exec_p.bind(
            *operands,
            out_avals=tuple(out_avals),
            in_names=tuple(all_names),
            out_names=tuple(out_names),
            lowering_input_output_aliases=(),
            sim_require_finite=True,
            sim_require_nnan=True,
            nc=nc,
        )
        return tuple(outs)

    devices = jax.devices()[:NCORES]
    mesh = Mesh(np.asarray(devices), ("core",))
    n_args = n_params + len(out_names)
    jitted = jax.jit(
        shard_map(
            _body,
            mesh=mesh,
            in_specs=(PartitionSpec("core"),) * n_args,
            out_specs=(PartitionSpec("core"),) * len(out_names),
            check_rep=False,
        ),
        keep_unused=True,
    )
    _RUNNER = (jitted, in_names, out_names, n_params, zero_outs)
    return _RUNNER


def _run(in_maps, time_iters=0):
    """Execute on 8 cores. Returns (results_list, min_wall_ns or None)."""
    import jax
    from jax.sharding import Mesh, PartitionSpec, NamedSharding

    jitted, in_names, out_names, n_params, zero_outs = _get_runner()
    concat_in = [
        np.concatenate([m[name] for m in in_maps], axis=0) for name in in_names
    ] + [
        np.concatenate([z] * NCORES, axis=0) for z in zero_outs
    ]
    devices = jax.devices()[:NCORES]
    mesh = Mesh(np.asarray(devices), ("core",))
    shard = NamedSharding(mesh, PartitionSpec("core"))
    dev_in = jax.device_put(concat_in, [shard] * len(concat_in))
    outs = jitted(*dev_in)
    jax.block_until_ready(outs)

    best_ns = None
    if time_iters > 0:
        import time as _time

        for _ in range(time_iters):
            t0 = _time.perf_counter()
            outs2 = jitted(*dev_in)
            jax.block_until_ready(outs2)
            dt = (_time.perf_counter() - t0) * 1e9
            best_ns = dt if best_ns is None else min(best_ns, dt)
        outs = outs2

    results = []
    np_outs = [np.asarray(o) for o in outs]
    per_core = NJ  # axis-0 length of each core's "out"
    for c in range(NCORES):
        results.append(
            {
                name: np_outs[i][c * per_core:(c + 1) * per_core]
                for i, name in enumerate(out_names)
            }
        )
    return results, best_ns


def _assemble(results):
    out = np.empty((B, N, HC), dtype=np.float32)
    for c in range(NCORES):
        b, half = divmod(c, 2)
        j0 = half * NJ
        out[b, j0:j0 + NJ, :] = results[c]["out"]
    return out


def kernel(features_batch, adj_mats_batch, W, att_src, att_dst, bias):
    in_maps = _make_in_maps(
        features_batch, adj_mats_batch, W, att_src, att_dst, bias
    )
    results, _ = _run(in_maps)
    return _assemble(results)


def run_profiled(features_batch, adj_mats_batch, W, att_src, att_dst, bias,
                 time_iters=10):
    """Like kernel() but also times warm executions; returns (out, min_ns)."""
    in_maps = _make_in_maps(
        features_batch, adj_mats_batch, W, att_src, att_dst, bias
    )
    results, best_ns = _run(in_maps, time_iters=time_iters)
    return _assemble(results), best_ns
